# revision 3
# baseline (speedup 1.0000x reference)
"""Trainium2 Bass kernel (v8) for FFTConv: y = tanh(ifft(fft(u)*fft(k)).real + diag(D)*u).

Shapes: u (8,256,16384) f32, k (256,16384) f32, D (256,256) f32.
Measured (repeat-loop delta on HW): ~199 us vs 274 us for the previous
kernel; TimelineSim estimate 167 us.

Design (H sharded 8 ways, 32 channels/core; per (b,h) the length-16384
circular conv runs as a 128x128 four-step FFT on the tensor engine, with
pairs of batch rows packed as re/im of one complex FFT):
- All DRAM tensors p-major ([128, HSH*cols]) so every DMA is a plain 2D
  slice with 8-16KB contiguous runs; 8-channel batched transfers issued in
  staggered halves (~12 big DMAs); constants ride the Activation-engine DGE
  queue so they don't block the input stream.
- khat is 4 MiB/core: [s(2), mm(2), d(2), c] = [Kr|Ki|Kr|Ki | Ki|Kr|Ki|Kr];
  spectrum multiply = 4 DVE ops [128,512]; diag(D) folded into k[0].
- The spectrum combine (ptcomb) is folded into the inv1 matmuls via
  sign-folded moving constants f2mov = [A|-A|B]: 16 matmuls accumulate
  q[0,m,0]@A - q[0,m,1]@A + q[1,m,1]@B + q[1,m,0]@B directly in PSUM.
- Engine assignment (HW-measured rates; gpsimd degrades ~2-3x under full
  load so Pool gets exactly ONE op per channel):
    PE   : fwd1 8mm, fwd2 6mm, inv1 16mm (with combine), inv2 6mm
    DVE  : twiddle mul-a + sub, spectrum muls, inv-twiddle muls + add
    Pool : twiddle mul-b only
    Act  : dt/et/g PSUM evacuations + tanh
- Software-pipelined slot loop (DEPTH=11), PSUM [128,1024] bufs=1 per stage.
- f16 I/O both ways (host casts/transposes); rel err ~1.16e-2 vs gate 2e-2.
"""

import numpy as np

B, H, L = 8, 256, 16384
N = 128
HSH = H // 8   # 32 channels per core
NP = B // 2    # 4 packed pairs per channel
GRP = 8        # channels per DMA group

_CACHE = {}


def _consts():
    n = np.arange(N)
    F1 = np.exp(-2j * np.pi * np.outer(n, n) / N)
    F1r = F1.real.astype(np.float32)
    F1i = F1.imag.astype(np.float32)
    T = np.exp(-2j * np.pi * np.outer(n, n) / L)
    Tr = T.real.astype(np.float32)
    Ti = T.imag.astype(np.float32)
    f16 = lambda x: np.ascontiguousarray(x).astype(np.float16)
    t4 = lambda a, b: np.tile(np.concatenate([a, b], 1), (1, 4))
    c = {}
    # FWD1 moving consts (f16): [F1r|F1i], [-F1i|F1r]
    c["f1a"] = f16(np.concatenate([F1r, F1i], 1))
    c["f1b"] = f16(np.concatenate([-F1i, F1r], 1))
    # FWD2 stationaries (f16): [F2r|F2i|-F2i]
    c["f2s"] = f16(np.concatenate([F1r, F1i, -F1i], 1))
    # fwd twiddle, sign-folded: [tile([Tr|Ti],4) | tile([Ti|-Tr],4)]
    c["ttab"] = f16(np.concatenate([t4(Tr, Ti), t4(Ti, -Tr)], 1))
    # INV1 moving consts (f16): [F2r|-F2i | -F2r|F1i... ] = [A|-A|B]
    c["f2mov"] = f16(np.concatenate([F1r, -F1i, -F1r, F1i, F1i, F1r], 1))
    # inv twiddle (conj), scaled 1/N: [tile([Tr|Ti],4) | tile([-Ti|Tr],4)]/N
    c["tiab"] = f16(np.concatenate([t4(Tr, Ti), t4(-Ti, Tr)], 1) / N)
    # INV2 stationaries (f16): [F1r|F1i|-F1i]
    c["f1s3"] = f16(np.concatenate([F1r, F1i, -F1i], 1))
    return c


def _build_nc(repeat=1):
    import concourse.bass as bass  # noqa: F401
    import concourse.mybir as mybir
    import concourse.tile as tile
    from concourse import bacc

    F32, F16 = mybir.dt.float32, mybir.dt.float16
    MUL = mybir.AluOpType.mult
    SUB = mybir.AluOpType.subtract
    ADD = mybir.AluOpType.add
    COPY = mybir.ActivationFunctionType.Copy
    TANH = mybir.ActivationFunctionType.Tanh

    nc = bacc.Bacc("TRN2", target_bir_lowering=False, debug=False,
                   num_devices=8)

    u_d = nc.dram_tensor("u", [N, HSH * 1024], F16, kind="ExternalInput")
    khat_d = nc.dram_tensor("khat", [N, HSH * 1024], F16, kind="ExternalInput")
    f1a_d = nc.dram_tensor("f1a", [N, 256], F16, kind="ExternalInput")
    f1b_d = nc.dram_tensor("f1b", [N, 256], F16, kind="ExternalInput")
    f2s_d = nc.dram_tensor("f2s", [N, 384], F16, kind="ExternalInput")
    ttab_d = nc.dram_tensor("ttab", [N, 2048], F16, kind="ExternalInput")
    f2mov_d = nc.dram_tensor("f2mov", [N, 768], F16, kind="ExternalInput")
    tiab_d = nc.dram_tensor("tiab", [N, 2048], F16, kind="ExternalInput")
    f1s3_d = nc.dram_tensor("f1s3", [N, 384], F16, kind="ExternalInput")
    y_d = nc.dram_tensor("y", [N, HSH * 1024], F16, kind="ExternalOutput")

    from contextlib import ExitStack, nullcontext

    with tile.TileContext(nc) as tc:
        with ExitStack() as stack:
            ep = stack.enter_context
            cp = ep(tc.tile_pool(name="const", bufs=1))
            pu = ep(tc.tile_pool(name="u4", bufs=2))
            pkh = ep(tc.tile_pool(name="kh4", bufs=2))
            py4 = ep(tc.tile_pool(name="y4", bufs=2))
            pdt = ep(tc.tile_pool(name="dt16", bufs=3))
            pm12 = ep(tc.tile_pool(name="m12", bufs=2))
            pct = ep(tc.tile_pool(name="ct", bufs=3))
            pet = ep(tc.tile_pool(name="et16", bufs=3))
            pq12 = ep(tc.tile_pool(name="q12", bufs=3))
            ppt = ep(tc.tile_pool(name="pt", bufs=3))
            pg = ep(tc.tile_pool(name="g16", bufs=3))
            pr12 = ep(tc.tile_pool(name="r12", bufs=2))
            ph16 = ep(tc.tile_pool(name="h16", bufs=3))
            # PSUM: dt/et/g/y as [128,1024] f32 (2 banks each), bufs=1.
            pdt_ps = ep(tc.tile_pool(name="dtps", bufs=1, space="PSUM"))
            pet_ps = ep(tc.tile_pool(name="etps", bufs=1, space="PSUM"))
            pg_ps = ep(tc.tile_pool(name="gps", bufs=1, space="PSUM"))
            py_ps = ep(tc.tile_pool(name="yps", bufs=1, space="PSUM"))

            # ---- constants ----
            c_f1a = cp.tile([N, 256], F16)
            nc.scalar.dma_start(c_f1a[:], f1a_d[:])
            c_f1b = cp.tile([N, 256], F16)
            nc.scalar.dma_start(c_f1b[:], f1b_d[:])
            c_f2s = cp.tile([N, 384], F16)
            nc.scalar.dma_start(c_f2s[:], f2s_d[:])
            c_ttab = cp.tile([N, 2048], F16)
            nc.scalar.dma_start(c_ttab[:], ttab_d[:])
            c_f2mov = cp.tile([N, 768], F16)
            nc.scalar.dma_start(c_f2mov[:], f2mov_d[:])
            c_tiab = cp.tile([N, 2048], F16)
            nc.scalar.dma_start(c_tiab[:], tiab_d[:])
            c_f1s3 = cp.tile([N, 384], F16)
            nc.scalar.dma_start(c_f1s3[:], f1s3_d[:])

            rep_ctx = tc.For_i(0, repeat, 1) if repeat > 1 else nullcontext()
            stack.enter_context(rep_ctx)

            tiles = {}

            # ---- stage emitters -------------------------------------------
            HGRP = GRP * 1024 // 2

            def st_dma_in_a(h):
                g = h // GRP
                u4 = pu.tile([N, GRP * 1024], F16)
                kh4 = pkh.tile([N, GRP * 1024], F16)
                o = g * GRP * 1024
                nc.sync.dma_start(u4[:, 0:HGRP], u_d[:, o:o + HGRP])
                tiles[("u4", g)] = u4
                tiles[("kh4", g)] = kh4

            def st_dma_in_b(h):
                g = h // GRP
                u4 = tiles[("u4", g)]
                kh4 = tiles[("kh4", g)]
                o = g * GRP * 1024
                nc.sync.dma_start(u4[:, HGRP:2 * HGRP],
                                  u_d[:, o + HGRP:o + 2 * HGRP])
                nc.sync.dma_start(kh4[:, 0:HGRP], khat_d[:, o:o + HGRP])

            def st_dma_in_c(h):
                g = h // GRP
                kh4 = tiles[("kh4", g)]
                o = g * GRP * 1024
                nc.sync.dma_start(kh4[:, HGRP:2 * HGRP],
                                  khat_d[:, o + HGRP:o + 2 * HGRP])

            def st_fwd1(h):
                u4 = tiles[("u4", h // GRP)]
                co = (h % GRP) * 1024
                dt_ps = pdt_ps.tile([N, 1024], F32)
                for m in range(NP):
                    o = m * 256
                    nc.tensor.matmul(dt_ps[:, o:o + 256],
                                     u4[:, co + (2 * m) * N:co + (2 * m + 1) * N],
                                     c_f1a[:], start=(m % 2 == 0), stop=False)
                    nc.tensor.matmul(dt_ps[:, o:o + 256],
                                     u4[:, co + (2 * m + 1) * N:co + (2 * m + 2) * N],
                                     c_f1b[:], start=False, stop=(m % 2 == 1))
                tiles[("dtps", h)] = dt_ps

            def st_dt_evac(h):
                dt_ps = tiles.pop(("dtps", h))
                dt16 = pdt.tile([N, 1024], F16)
                nc.scalar.activation(dt16[:], dt_ps[:], COPY)
                tiles[("dt16", h)] = dt16

            def st_tw_mul(h):
                dt16 = tiles.pop(("dt16", h))
                m12 = pm12.tile([N, 2048], F16)
                nc.vector.tensor_tensor(m12[:, 0:1024], dt16[:],
                                        c_ttab[:, 0:1024], MUL)
                nc.gpsimd.tensor_tensor(m12[:, 1024:2048], dt16[:],
                                        c_ttab[:, 1024:2048], MUL)
                tiles[("m12", h)] = m12

            def st_tw_sub(h):
                m12 = tiles.pop(("m12", h))
                ct = pct.tile([N, 1024], F16)
                v = m12[:].rearrange("p (t m d c) -> p m t d c", t=2, m=NP, d=2)
                ctv = ct[:].rearrange("p (m t c) -> p m t c", m=NP, t=2)
                nc.vector.tensor_tensor(ctv, v[:, :, :, 0, :], v[:, :, :, 1, :],
                                        SUB)
                tiles[("ct", h)] = ct

            def st_fwd2(h):
                ct = tiles.pop(("ct", h))
                et_ps = pet_ps.tile([N, 1024], F32)
                ctv = ct[:].rearrange("p (m t c) -> p m t c", m=NP, t=2)
                etv = et_ps[:].rearrange("p (m t c) -> p m t c", m=NP, t=2)
                for half in range(2):
                    mm = slice(2 * half, 2 * half + 2)
                    o = half * 512
                    nc.tensor.matmul(et_ps[:, o:o + 512], c_f2s[:, 0:N],
                                     ct[:, o:o + 512], start=True, stop=False)
                    nc.tensor.matmul(etv[:, mm, 0, :], c_f2s[:, 256:384],
                                     ctv[:, mm, 1, :], start=False, stop=False)
                    nc.tensor.matmul(etv[:, mm, 1, :], c_f2s[:, N:256],
                                     ctv[:, mm, 0, :], start=False, stop=True)
                tiles[("etps", h)] = et_ps

            def st_et_evac(h):
                et_ps = tiles.pop(("etps", h))
                et16 = pet.tile([N, 1024], F16)
                nc.scalar.activation(et16[:], et_ps[:], COPY)
                tiles[("et16", h)] = et16

            def st_qmul(h):
                et16 = tiles.pop(("et16", h))
                kh4 = tiles[("kh4", h // GRP)]
                ko = (h % GRP) * 1024
                q12 = pq12.tile([N, 2048], F16)
                # q12 cols [s(2), m(4), d(2), c]; khat cols [s(2), mm(2), d, c]
                for s in range(2):
                    for w in range(2):
                        nc.vector.tensor_tensor(
                            q12[:, s * 1024 + w * 512:s * 1024 + w * 512 + 512],
                            et16[:, w * 512:w * 512 + 512],
                            kh4[:, ko + s * 512:ko + s * 512 + 512], MUL)
                tiles[("q12", h)] = q12

            def st_inv1(h):
                q12 = tiles.pop(("q12", h))
                g_ps = pg_ps.tile([N, 1024], F32)
                # q12 cols [s(2), m(4), d(2), c(128)]; g accumulates
                # q[0,m,0]@A - q[0,m,1]@A + q[1,m,1]@B + q[1,m,0]@B
                # via f2mov = [A | -A | B].
                def qb(s_, m_, d_):
                    o = s_ * 1024 + m_ * 256 + d_ * 128
                    return q12[:, o:o + 128]
                for m in range(NP):
                    o = m * 256
                    nc.tensor.matmul(g_ps[:, o:o + 256], qb(0, m, 0),
                                     c_f2mov[:, 0:256],
                                     start=(m % 2 == 0), stop=False)
                    nc.tensor.matmul(g_ps[:, o:o + 256], qb(0, m, 1),
                                     c_f2mov[:, 256:512],
                                     start=False, stop=False)
                    nc.tensor.matmul(g_ps[:, o:o + 256], qb(1, m, 1),
                                     c_f2mov[:, 512:768],
                                     start=False, stop=False)
                    nc.tensor.matmul(g_ps[:, o:o + 256], qb(1, m, 0),
                                     c_f2mov[:, 512:768],
                                     start=False, stop=(m % 2 == 1))
                tiles[("gps", h)] = g_ps

            def st_g_evac(h):
                g_ps = tiles.pop(("gps", h))
                g16 = pg.tile([N, 1024], F16)
                nc.scalar.activation(g16[:], g_ps[:], COPY)
                tiles[("g16", h)] = g16

            def st_invtw_mul(h):
                g16 = tiles.pop(("g16", h))
                r12 = pr12.tile([N, 2048], F16)
                nc.vector.tensor_tensor(r12[:, 0:1024], g16[:],
                                        c_tiab[:, 0:1024], MUL)
                nc.vector.tensor_tensor(r12[:, 1024:2048], g16[:],
                                        c_tiab[:, 1024:2048], MUL)
                tiles[("r12", h)] = r12

            def st_inv_add(h):
                r12 = tiles.pop(("r12", h))
                h16 = ph16.tile([N, 1024], F16)
                v = r12[:].rearrange("p (t m d c) -> p m t d c", t=2, m=NP, d=2)
                hv = h16[:].rearrange("p (m t c) -> p m t c", m=NP, t=2)
                nc.vector.tensor_tensor(hv, v[:, :, :, 0, :], v[:, :, :, 1, :],
                                        ADD)
                tiles[("h16", h)] = h16

            def st_inv2(h):
                h16 = tiles.pop(("h16", h))
                y_ps = py_ps.tile([N, 1024], F32)
                hv = h16[:].rearrange("p (m t c) -> p m t c", m=NP, t=2)
                yv = y_ps[:].rearrange("p (m t c) -> p m t c", m=NP, t=2)
                for half in range(2):
                    mm = slice(2 * half, 2 * half + 2)
                    o = half * 512
                    nc.tensor.matmul(y_ps[:, o:o + 512], c_f1s3[:, 0:N],
                                     h16[:, o:o + 512], start=True, stop=False)
                    nc.tensor.matmul(yv[:, mm, 0, :], c_f1s3[:, N:256],
                                     hv[:, mm, 1, :], start=False, stop=False)
                    nc.tensor.matmul(yv[:, mm, 1, :], c_f1s3[:, 256:384],
                                     hv[:, mm, 0, :], start=False, stop=True)
                tiles[("yps", h)] = y_ps

            def st_tanh(h):
                y_ps = tiles.pop(("yps", h))
                if h % GRP == 0:
                    y4n = py4.tile([N, GRP * 1024], F16)
                    tiles[("y4", h // GRP)] = y4n
                y4 = tiles[("y4", h // GRP)]
                o = (h % GRP) * 1024
                nc.scalar.activation(y4[:, o:o + 1024], y_ps[:], TANH)

            def st_dma_out_a(h):
                g = (h - (GRP // 2 - 1)) // GRP
                y4 = tiles[("y4", g)]
                o = g * GRP * 1024
                nc.sync.dma_start(y_d[:, o:o + HGRP], y4[:, 0:HGRP])

            def st_dma_out_b(h):
                g = (h - (GRP - 1)) // GRP
                y4 = tiles.pop(("y4", g))
                o = g * GRP * 1024
                nc.sync.dma_start(y_d[:, o + HGRP:o + 2 * HGRP],
                                  y4[:, HGRP:2 * HGRP])

            # ---- software-pipelined slot loop -----------------------------
            DEPTH = 11
            for i in range(HSH + DEPTH):
                def live(o, mod=None, rem=0):
                    hh = i - o
                    if 0 <= hh < HSH and (mod is None or hh % mod == rem):
                        return hh
                    return None

                if (h := live(0, GRP, 0)) is not None:
                    st_dma_in_a(h)
                if (h := live(0, GRP, 2)) is not None:
                    st_dma_in_b(h)
                if (h := live(0, GRP, 4)) is not None:
                    st_dma_in_c(h)
                if (h := live(1)) is not None:
                    st_fwd1(h)
                    st_dt_evac(h)
                if (h := live(2)) is not None:
                    st_tw_mul(h)
                if (h := live(3)) is not None:
                    st_tw_sub(h)
                if (h := live(4)) is not None:
                    st_fwd2(h)
                    st_et_evac(h)
                if (h := live(5)) is not None:
                    st_qmul(h)
                if (h := live(7)) is not None:
                    st_inv1(h)
                    st_g_evac(h)
                if (h := live(8)) is not None:
                    st_invtw_mul(h)
                if (h := live(9)) is not None:
                    st_inv_add(h)
                if (h := live(10)) is not None:
                    st_inv2(h)
                    st_tanh(h)
                if (h := live(11, GRP, GRP // 2 - 1)) is not None:
                    st_dma_out_a(h)
                if (h := live(11, GRP, GRP - 1)) is not None:
                    st_dma_out_b(h)

    nc.finalize()
    return nc


def make_in_maps(u, k, D):
    u = np.ascontiguousarray(u, dtype=np.float32)
    k = np.ascontiguousarray(k, dtype=np.float32)
    D = np.ascontiguousarray(D, dtype=np.float32)

    c = _consts()
    k2 = k.copy()
    k2[:, 0] += np.diag(D)
    Kf = np.fft.fft(k2, axis=-1).reshape(H, N, N) / N
    Kr = Kf.real.astype(np.float16)
    Ki = Kf.imag.astype(np.float16)
    A = np.concatenate([Kr, Ki], axis=2)            # [Kr|Ki]
    Bm = np.concatenate([Ki, Kr], axis=2)           # [Ki|Kr]
    khat = np.concatenate([np.tile(A, (1, 1, 2)),
                           np.tile(Bm, (1, 1, 2))], axis=2)  # (H, 128, 1024)

    # u host-relayout: (B, H, L) -> p-major [p, h, b, c] per core
    u16 = u.astype(np.float16).reshape(B, H, N, N)
    u_pm = u16.transpose(2, 1, 0, 3)                # (N, H, B, N)
    khat_pm = khat.transpose(1, 0, 2)               # (N, H, 1024)

    in_maps = []
    for core in range(8):
        h0 = core * HSH
        m = {
            "u": np.ascontiguousarray(
                u_pm[:, h0:h0 + HSH]).reshape(N, HSH * 1024),
            "khat": np.ascontiguousarray(
                khat_pm[:, h0:h0 + HSH]).reshape(N, HSH * 1024),
        }
        for name in ("f1a", "f1b", "f2s", "ttab", "f2mov", "tiab", "f1s3"):
            m[name] = c[name]
        in_maps.append(m)
    return in_maps


def kernel(u, k, D, **_ignore):
    from concourse.bass_utils import run_bass_kernel_spmd

    if "nc" not in _CACHE:
        _CACHE["nc"] = _build_nc()
    nc = _CACHE["nc"]

    in_maps = make_in_maps(u, k, D)
    res = run_bass_kernel_spmd(nc, in_maps, core_ids=list(range(8)),
                               **_CACHE.get("run_kwargs", {}))
    _CACHE["last_result"] = res
    # y per core: [N, HSH*1024] = [p, h, b, c] -> (B, HSH, L)
    ys = []
    for core in range(8):
        yc = res.results[core]["y"].reshape(N, HSH, B, N)
        ys.append(yc.transpose(2, 1, 0, 3).reshape(B, HSH, L))
    y = np.concatenate(ys, axis=1)
    return y.astype(np.float32)
